# revision 7
# baseline (speedup 1.0000x reference)
"""Inverse discrete Hough transform on 8 Trainium2 NeuronCores — v6.

v5 + mixed-shape tiling: angles are split into two sets; set V is
processed with 16x8 pixel tiles, set H with 8x16 tiles. A tile's rho
band per angle has width ~ (tw-1)|cos| + (th-1)|sin| + 1, so matching
the tile aspect to the angle shrinks the gathered row count (moving
operand bytes and PE passes) by ~13%. Each tiling produces a partial
image (V-angle sum / H-angle sum); the two bf16 partials are summed on
the host. The angle split is tuned per-tile-budget to minimize total
128-row passes including padding.
"""
import sys, os

sys.path.insert(0, "/opt/trn_rl_repo")
import numpy as np
import ml_dtypes

from concourse import bass, tile
from concourse.bass_utils import run_bass_kernel_spmd
import concourse.mybir as mybir

OUT_H = 256
OUT_W = 256
NUMANGLE = 180
NUMRHO = 400
N_B, C_CH = 4, 64
NCH = N_B * C_CH
N_CORES = 8
ROWS_PER_CORE = OUT_H // N_CORES  # 32

f32 = mybir.dt.float32
bf16 = mybir.dt.bfloat16
fp8 = mybir.dt.float8e3
np_fp8 = ml_dtypes.float8_e3m4

_MAX_INSTR_WAITS = 1


def _split_excess_waits(nc):
    n = 0
    for fn in nc.m.functions:
        for bb in fn.blocks:
            out = []
            changed = False
            for inst in bb.instructions:
                si = inst.sync_info
                waits = list(si.on_wait) if si and si.on_wait else []
                if len(waits) > _MAX_INSTR_WAITS:
                    for w in waits[_MAX_INSTR_WAITS:]:
                        nop = mybir.InstNoOp(
                            name=f"waitsplit-{n}-{inst.name}", ins=[], outs=[]
                        )
                        n += 1
                        nop.engine = inst.engine
                        nop.sync_info = mybir.SyncInfo(on_wait=[w], on_update=[])
                        out.append(nop)
                    inst.sync_info = mybir.SyncInfo(
                        on_wait=waits[:_MAX_INSTR_WAITS],
                        on_update=list(si.on_update or []),
                    )
                    changed = True
                out.append(inst)
            if changed:
                bb.instructions = out
    return n


def _install_ntff_hook():
    import types
    import antenv

    if hasattr(antenv, "axon_hooks"):
        return
    try:
        from trn_agent_boot.trn_boot import _ntff_profile_via_ctypes
    except ImportError:
        return
    hook = _ntff_profile_via_ctypes("/opt/axon/libaxon_pjrt.so")
    mod = types.ModuleType("antenv.axon_hooks")
    mod.get_axon_ntff_profile_hook = lambda: hook
    mod.set_axon_ntff_profile_hook = lambda h: None
    sys.modules["antenv.axon_hooks"] = mod
    antenv.axon_hooks = mod


_install_ntff_hook()


def _rho_index_table():
    """Mirror of the reference's jnp fp32 math (jax preferred so rounding
    matches the harness bit-for-bit; numpy fp32 fallback)."""
    try:
        import jax
        import jax.numpy as jnp

        with jax.default_device(jax.devices("cpu")[0]):
            angles = jnp.arange(NUMANGLE, dtype=jnp.float32) * (np.pi / NUMANGLE)
            cos_t = jnp.cos(angles)
            sin_t = jnp.sin(angles)
            xs = (jnp.arange(OUT_W) - OUT_W // 2).astype(jnp.float32)
            ys = (jnp.arange(OUT_H) - OUT_H // 2).astype(jnp.float32)
            r = jnp.round(
                xs[None, None, :] * cos_t[:, None, None]
                + ys[None, :, None] * sin_t[:, None, None]
            ).astype(jnp.int32) + NUMRHO // 2
            valid = (r >= 0) & (r < NUMRHO)
            r = jnp.clip(r, 0, NUMRHO - 1)
            return np.asarray(r), np.asarray(valid)
    except Exception:
        angles = (np.arange(NUMANGLE, dtype=np.float32) * np.float32(np.pi / NUMANGLE)).astype(np.float32)
        cos_t = np.cos(angles).astype(np.float32)
        sin_t = np.sin(angles).astype(np.float32)
        xs = (np.arange(OUT_W) - OUT_W // 2).astype(np.float32)
        ys = (np.arange(OUT_H) - OUT_H // 2).astype(np.float32)
        z = (
            xs[None, None, :] * cos_t[:, None, None]
            + ys[None, :, None] * sin_t[:, None, None]
        )
        r = np.round(z).astype(np.int32) + NUMRHO // 2
        valid = (r >= 0) & (r < NUMRHO)
        r = np.clip(r, 0, NUMRHO - 1)
        return r, valid


_STATIC = {}


def _tile_defs(shape, y0):
    """Unit list for one core: (ys, xs, th, tw) covering [y0,y0+32)x[0,256)."""
    th, tw = shape
    out = []
    for ty in range(ROWS_PER_CORE // th):
        for tx in range(OUT_W // tw):
            out.append((y0 + ty * th, tx * tw, th, tw))
    return out


def _build_static():
    if _STATIC:
        return _STATIC
    r_idx, valid = _rho_index_table()  # [A, H, W]

    # exact mean band width per angle for each shape (over core-0 tiles,
    # geometry is y-translation-covariant enough for the split choice)
    ang = np.arange(NUMANGLE)
    cs = np.abs(np.cos(ang * np.pi / NUMANGLE))
    sn = np.abs(np.sin(ang * np.pi / NUMANGLE))
    wV = 7 * cs + 15 * sn + 1.0   # 16x8 tile
    wH = 15 * cs + 7 * sn + 1.0   # 8x16 tile

    # tuned split: angle -> V if wV - wH < tau; pick tau minimizing
    # estimated total passes (incl. 128-padding) per tile
    best = None
    for tau in np.linspace(-4, 4, 33):
        selV = (wV - wH) < tau
        rv = wV[selV].sum()
        rh = wH[~selV].sum()
        cost = np.ceil(rv / 128.0) + np.ceil(rh / 128.0)
        key = (cost, abs(rv - rh))
        if best is None or key < best[0]:
            best = (key, selV)
    selV = best[1]
    V_ANGLES = np.where(selV)[0]
    H_ANGLES = np.where(~selV)[0]
    _STATIC["V_ANGLES"], _STATIC["H_ANGLES"] = V_ANGLES, H_ANGLES

    def build_units(core):
        y0 = core * ROWS_PER_CORE
        units = []
        for shape, angles in (((16, 8), V_ANGLES), ((8, 16), H_ANGLES)):
            for (ys_, xs_, th, tw) in _tile_defs(shape, y0):
                tr = r_idx[angles, ys_ : ys_ + th, xs_ : xs_ + tw].reshape(
                    len(angles), th * tw
                )
                tv = valid[angles, ys_ : ys_ + th, xs_ : xs_ + tw].reshape(
                    len(angles), th * tw
                )
                lo = tr.min(axis=1)
                hi = tr.max(axis=1)
                widths = hi - lo + 1
                L = int(widths.sum())
                rows = np.zeros((L, 2), np.int32)
                pos = 0
                for i, a in enumerate(angles):
                    w = widths[i]
                    rows[pos : pos + w, 0] = a
                    rows[pos : pos + w, 1] = np.arange(lo[i], hi[i] + 1)
                    pos += w
                units.append((rows, tr, tv, angles, (ys_, xs_, th, tw)))
        return units

    per_core = [build_units(c) for c in range(N_CORES)]
    NU = len(per_core[0])  # 128 units
    P_us = [
        max((len(per_core[c][u][0]) + 127) // 128 for c in range(N_CORES))
        for u in range(NU)
    ]
    TOTP = sum(P_us)
    _STATIC["P_us"] = P_us
    _STATIC["TOTP"] = TOTP
    _STATIC["NU"] = NU
    _STATIC["unit_geom"] = [per_core[0][u][4] for u in range(NU)]

    rowidx_cores = []
    oh_cores = []
    for core in range(N_CORES):
        rowidx = np.zeros((128, TOTP), np.int64)
        oh = np.zeros((128, TOTP, 128), np.float32)
        off = 0
        for u in range(NU):
            rows, tr, tv, angles, geom = per_core[core][u]
            P_u = P_us[u]
            L = len(rows)
            a_arr = rows[:, 0]
            rho_arr = rows[:, 1]
            # map absolute angle id -> index within this unit's angle set
            a_pos = np.searchsorted(angles, a_arr)
            blk_idx = np.zeros((P_u * 128,), np.int64)
            blk_idx[:L] = a_arr.astype(np.int64) * NUMRHO + rho_arr
            blk_oh = np.zeros((P_u * 128, 128), np.float32)
            blk_oh[:L] = (tr[a_pos] == rho_arr[:, None]) & tv[a_pos]
            rowidx[:, off : off + P_u] = blk_idx.reshape(P_u, 128).T
            oh[:, off : off + P_u] = blk_oh.reshape(P_u, 128, 128).transpose(1, 0, 2)
            off += P_u
        rowidx_cores.append(rowidx)
        oh_cores.append(np.ascontiguousarray(oh).astype(np_fp8))

    _STATIC["rowidx"] = rowidx_cores
    _STATIC["oh"] = oh_cores
    return _STATIC


_PROGRAM = {}


def _build_program(P_us, TOTP, NU):
    if "nc" in _PROGRAM:
        return _PROGRAM["nc"]
    nc = bass.Bass()
    mov_dram = nc.declare_dram_parameter("mov", [128, TOTP, NCH], fp8, isOutput=False)
    oh_dram = nc.declare_dram_parameter("oh", [128, TOTP, 128], fp8, isOutput=False)
    out_dram = nc.declare_dram_parameter("out", [128, NU, NCH], bf16, isOutput=True)

    # small groups at head (fast PE start) and tail (short drain)
    head, tail = [1, 1, 2], [2, 1, 1]
    mid_n = NU - sum(head) - sum(tail)
    sched = head + [4] * (mid_n // 4) + ([mid_n % 4] if mid_n % 4 else []) + tail
    groups = []
    t = 0
    for gs in sched:
        if t >= NU:
            break
        groups.append(list(range(t, min(t + gs, NU))))
        t += gs
    assert sum(len(g) for g in groups) == NU

    with tile.TileContext(nc) as tc:
        with (
            tc.tile_pool(name="mov", bufs=4) as movp,
            tc.tile_pool(name="oh", bufs=4) as ohp,
            tc.tile_pool(name="out", bufs=4) as outp,
            tc.tile_pool(name="psum", bufs=8, space="PSUM") as psump,
        ):
            off = 0
            for gts in groups:
                g = gts[0]
                PG = sum(P_us[u] for u in gts)
                mov_sb = movp.tile([128, PG, NCH], fp8, tag="mov")
                nc.sync.dma_start(mov_sb[:], mov_dram[:, off : off + PG])
                oh_sb = ohp.tile([128, PG, 128], fp8, tag="oh")
                nc.sync.dma_start(oh_sb[:], oh_dram[:, off : off + PG])
                y = outp.tile([128, len(gts), NCH], bf16, tag="out")
                po = 0
                for i, u in enumerate(gts):
                    P_u = P_us[u]
                    acc_ps = psump.tile([128, NCH], f32, tag="psum")
                    for p in range(P_u):
                        nc.tensor.matmul(
                            acc_ps[:],
                            oh_sb[:, po + p, :],
                            mov_sb[:, po + p, :],
                            start=(p == 0),
                            stop=(p == P_u - 1),
                        )
                    nc.vector.tensor_copy(y[:, i], acc_ps[:])
                    po += P_u
                nc.scalar.dma_start(out_dram[:, g : g + len(gts)], y[:])
                off += PG

    _split_excess_waits(nc)
    _PROGRAM["nc"] = nc
    return nc


def _run(accumulator: np.ndarray, trace: bool = False):
    st = _build_static()
    P_us, TOTP, NU = st["P_us"], st["TOTP"], st["NU"]
    nc = _build_program(P_us, TOTP, NU)

    accT = np.ascontiguousarray(
        accumulator.transpose(2, 3, 0, 1)
    ).reshape(NUMANGLE * NUMRHO, NCH).astype(np_fp8)

    in_maps = []
    for core in range(N_CORES):
        mov = accT[st["rowidx"][core]]
        in_maps.append({"mov": mov, "oh": st["oh"][core]})

    res = run_bass_kernel_spmd(
        nc, in_maps, list(range(N_CORES)), trace=trace
    )

    geoms = st["unit_geom"]
    full = np.zeros((NCH, OUT_H, OUT_W), np.float32)
    for core in range(N_CORES):
        y0 = core * ROWS_PER_CORE
        oc = np.asarray(res.results[core]["out"]).astype(np.float32)  # [128, NU, NCH]
        for u, (ys_, xs_, th, tw) in enumerate(geoms):
            blk = oc[:, u, :].reshape(th, tw, NCH).transpose(2, 0, 1)
            full[:, y0 + ys_ : y0 + ys_ + th, xs_ : xs_ + tw] += blk
    out = full.reshape(N_B, C_CH, OUT_H, OUT_W).astype(np.float32)
    return out, res


def kernel(accumulator: np.ndarray) -> np.ndarray:
    out, _ = _run(np.asarray(accumulator, dtype=np.float32), trace=False)
    return out
